# revision 15
# baseline (speedup 1.0000x reference)
"""Trainium2 Bass kernel for BertForSentenceClassification head.

Full (unsharded) inputs in, full outputs out. Internally: data-parallel over
the batch axis across 8 NeuronCores (8 samples per core), head parameters
replicated, final loss reduction on host.

Per-core pipeline (all activations kept feature-major / "transposed" so that
biases and LN affine params are per-partition operands):
  1. segment-mean pooling as matmul: reps[32,768] per sample
     (pooling matrix built on host from input_ids, pre-divided by segment
     counts), PE-transposed into repsT [768, 256].
  2. encoder: z = enc_W @ repsT + b; LN (stats via ones-matmul over the
     partition axis + K=1 broadcast matmul); exact GELU on ACT.
  3. per-sample MHA: q/k per head [96, 256]; scores packed 4 heads per PSUM
     tile; softmax on free axis; probs PE-transposed; AV with per-sample v
     tiles (repartitioned via SBUF->SBUF DMA); per-head out-proj contraction.
  4. classifier: LN + GELU + 2-class logits; CE loss partial sum on device.
Matmuls run in float32r (tf32-like, 11-bit mantissa, full PE rate for moving
dim >= 256). DRAM tensors feeding matmuls are declared float32r and receive
raw fp32 bits (PE rounds on read).
"""

import numpy as np

import concourse.bass as bass
import concourse.tile as tile
from concourse import bacc, mybir
from concourse.bass_utils import run_bass_kernel_spmd

B, L, H, S = 64, 1024, 768, 32
NH = 8
HD = H // NH                 # 96 head dim
SEP = 102
EPS_LN = 1e-5
NCORES = 8
BC = B // NCORES             # samples per core = 8
ROWS = BC * S                # activation columns per core = 256
SEQR = BC * L                # seq rows per core = 8192

FR = mybir.dt.float32r
F32 = mybir.dt.float32
AF = mybir.ActivationFunctionType

_PROGRAM = None  # cached across kernel() calls


def _build_program(debug=False):
    nc = bacc.Bacc()

    # ---------------- DRAM I/O ----------------
    seq = nc.dram_tensor("seq", [SEQR, H], FR, kind="ExternalInput")
    poolT = nc.dram_tensor("poolT", [SEQR, S], FR, kind="ExternalInput")
    wenc = nc.dram_tensor("wenc", [H, H], FR, kind="ExternalInput")
    wqk = nc.dram_tensor("wqk", [H, 2 * H], FR, kind="ExternalInput")
    wv = nc.dram_tensor("wv", [H, H], FR, kind="ExternalInput")
    wout = nc.dram_tensor("wout", [H, H], FR, kind="ExternalInput")
    wcls1 = nc.dram_tensor("wcls1", [H, H], FR, kind="ExternalInput")
    wcls2 = nc.dram_tensor("wcls2", [H, 2], FR, kind="ExternalInput")
    ident_d = nc.dram_tensor("ident", [128, 128], FR, kind="ExternalInput")
    ones_col_d = nc.dram_tensor("ones_col", [128, 2], FR, kind="ExternalInput")
    ones_row_d = nc.dram_tensor("ones_row", [1, 128], FR, kind="ExternalInput")
    vbias_d = nc.dram_tensor("vbias_bc", [128, H], F32, kind="ExternalInput")
    b_enc = nc.dram_tensor("b_enc", [H, 1], F32, kind="ExternalInput")
    b_qk = nc.dram_tensor("b_qk", [2 * H, 1], F32, kind="ExternalInput")
    b_out = nc.dram_tensor("b_out", [H, 1], F32, kind="ExternalInput")
    b_cls1 = nc.dram_tensor("b_cls1", [H, 1], F32, kind="ExternalInput")
    b_cls2 = nc.dram_tensor("b_cls2", [2, 1], F32, kind="ExternalInput")
    g_enc = nc.dram_tensor("g_enc", [H, 1], F32, kind="ExternalInput")
    be_enc = nc.dram_tensor("be_enc", [H, 1], F32, kind="ExternalInput")
    g_cls = nc.dram_tensor("g_cls", [H, 1], F32, kind="ExternalInput")
    be_cls = nc.dram_tensor("be_cls", [H, 1], F32, kind="ExternalInput")
    lab_d = nc.dram_tensor("lab", [ROWS, 1], F32, kind="ExternalInput")
    val_d = nc.dram_tensor("val", [ROWS, 1], F32, kind="ExternalInput")

    logits_out = nc.dram_tensor("logits_out", [ROWS, 2], F32, kind="ExternalOutput")
    loss_out = nc.dram_tensor("loss_out", [1, 2], F32, kind="ExternalOutput")
    dbg = {}
    if debug:
        for nm, shp in [("d_repsT", [H, ROWS]), ("d_xT", [H, ROWS]),
                        ("d_q0", [HD, ROWS]), ("d_k0", [HD, ROWS]),
                        ("d_v0", [S, H]), ("d_pT0", [S, ROWS]),
                        ("d_oT0", [HD, ROWS]), ("d_attT", [H, ROWS]),
                        ("d_hT", [H, ROWS]), ("d_z", [H, ROWS]),
                        ("d_zc", [H, ROWS])]:
            dbg[nm] = nc.dram_tensor(nm, shp, F32, kind="ExternalOutput")

    def tap_chunks(name, tiles):
        if debug and name in dbg:
            for k, t in enumerate(tiles):
                nc.sync.dma_start(
                    out=dbg[name][128 * k:128 * (k + 1), :],
                    in_=t[:].bitcast(F32),
                )

    def tap(name, t):
        if debug and name in dbg:
            nc.sync.dma_start(out=dbg[name][:], in_=t[:].bitcast(F32))

    HC = H // 128  # 6 feature chunks

    with tile.TileContext(nc) as tc, \
         tc.tile_pool(name="const", bufs=1) as const, \
         tc.tile_pool(name="wpool", bufs=1) as wpool, \
         tc.tile_pool(name="wmid", bufs=1) as wmid, \
         tc.tile_pool(name="acts", bufs=1) as acts:

        # ---- constants ----
        ident = const.tile([128, 128], FR, tag="ident")
        nc.sync.dma_start(out=ident[:], in_=ident_d[:])
        ones_col = const.tile([128, 2], FR, tag="ones_col")
        nc.sync.dma_start(out=ones_col[:], in_=ones_col_d[:])
        ones_row = const.tile([1, 128], FR, tag="ones_row")
        nc.sync.dma_start(out=ones_row[:], in_=ones_row_d[:])
        vbias = const.tile([128, H], F32, tag="vbias")
        nc.sync.dma_start(out=vbias[:], in_=vbias_d[:])

        def load_cols(dram, n_chunks, tag, p=128):
            # [n_chunks*p, 1] dram vector -> one [p, n_chunks] tile
            t = const.tile([p, n_chunks], F32, tag=tag)
            nc.sync.dma_start(
                out=t[:], in_=dram[:].rearrange("(c p) f -> p (c f)", p=p)
            )
            return t

        benc_t = load_cols(b_enc, HC, "benc")
        bqk_t = load_cols(b_qk, 2 * NH, "bqk", p=HD)       # [96, 16]
        bout_t = load_cols(b_out, HC, "bout")
        bcls1_t = load_cols(b_cls1, HC, "bcls1")
        genc_t = load_cols(g_enc, HC, "genc")
        beenc_t = load_cols(be_enc, HC, "beenc")
        gcls_t = load_cols(g_cls, HC, "gcls")
        becls_t = load_cols(be_cls, HC, "becls")
        bcls2_t = const.tile([2, 1], F32, tag="bcls2")
        nc.sync.dma_start(out=bcls2_t[:], in_=b_cls2[:])
        eps_t = const.tile([1, 1], F32, tag="eps")
        nc.vector.memset(eps_t[:], EPS_LN)
        lab_t = load_cols(lab_d, 2, "lab")                 # [128, 2]
        val_t = load_cols(val_d, 2, "val")

        # encoder weights early (needed right after pooling)
        wenc_t = [wpool.tile([128, H], FR, tag=f"wsq{k}", name=f"wsq{k}") for k in range(HC)]
        for k in range(HC):
            nc.sync.dma_start(out=wenc_t[k][:], in_=wenc[128 * k:128 * (k + 1), :])

        # ============ Stage 1: pooling + transpose to repsT ============
        repsT = [acts.tile([128, ROWS], FR, tag=f"ra{k}", name=f"ra{k}") for k in range(HC)]
        with tc.tile_pool(name="seqp", bufs=6) as seqp, \
             tc.tile_pool(name="poolp", bufs=6) as poolp, \
             tc.tile_pool(name="ps_rp", bufs=3, space="PSUM") as ps_rp, \
             tc.tile_pool(name="ps_tr", bufs=2, space="PSUM") as ps_tr, \
             tc.tile_pool(name="natp", bufs=3) as natp:
            for b in range(BC):
                ps = ps_rp.tile([S, H], F32, tag="ps_rp")
                for lc in range(8):
                    r0 = (b * 8 + lc) * 128
                    st = seqp.tile([128, H], FR, tag="seqt")
                    nc.sync.dma_start(out=st[:], in_=seq[r0:r0 + 128, :])
                    pt = poolp.tile([128, S], FR, tag="poolt")
                    nc.sync.dma_start(out=pt[:], in_=poolT[r0:r0 + 128, :])
                    for c0, c1 in ((0, 512), (512, 768)):
                        nc.tensor.matmul(
                            ps[:, c0:c1],
                            pt[:],
                            st[:, c0:c1],
                            start=(lc == 0), stop=(lc == 7),
                        )
                rn = natp.tile([S, H], FR, tag="nat")
                nc.vector.tensor_copy(out=rn[:], in_=ps[:])
                for k in range(HC):
                    pt2 = ps_tr.tile([128, S], FR, tag="ps_tr")
                    nc.tensor.transpose(
                        pt2[:], rn[:, 128 * k:128 * (k + 1)], ident[0:S, 0:S]
                    )
                    nc.scalar.copy(
                        out=repsT[k][:, S * b:S * (b + 1)], in_=pt2[:]
                    )

        # ============ helper: LN + GELU on transposed activations ============
        def ln_gelu(zs, g_cols, beta_cols, tag):
            """zs: 6 fr SBUF tiles [128, ROWS] (bias already added).
            Returns 6 fr SBUF tiles [128, ROWS] = gelu(LN(zs)*g + beta)."""
            out = [acts.tile([128, ROWS], FR, tag=f"xh{k}", name=f"xh{k}") for k in range(HC)]
            with tc.tile_pool(name=f"{tag}_st", bufs=1, space="PSUM") as stp, \
                 tc.tile_pool(name=f"{tag}_bc", bufs=1, space="PSUM") as bcp, \
                 tc.tile_pool(name=f"{tag}_sq", bufs=2) as sqp, \
                 tc.tile_pool(name=f"{tag}_sm", bufs=1) as smp:
                ps_sx = stp.tile([2, ROWS], F32, tag="ps_sx")
                ps_sq = stp.tile([2, ROWS], F32, tag="ps_sq")
                for k in range(HC):
                    sq = sqp.tile([128, ROWS], FR, tag="sq")
                    nc.vector.tensor_mul(
                        out=sq[:], in0=zs[k][:].bitcast(F32), in1=zs[k][:].bitcast(F32)
                    )
                    nc.tensor.matmul(ps_sx[:], ones_col[:], zs[k][:],
                                     start=(k == 0), stop=(k == HC - 1))
                    nc.tensor.matmul(ps_sq[:], ones_col[:], sq[:],
                                     start=(k == 0), stop=(k == HC - 1))
                mun = smp.tile([1, ROWS], F32, tag="mun")
                nc.vector.tensor_scalar_mul(mun[:], ps_sx[0:1, :], -1.0 / H)
                e2 = smp.tile([1, ROWS], F32, tag="e2")
                nc.vector.tensor_scalar_mul(e2[:], ps_sq[0:1, :], 1.0 / H)
                musq = smp.tile([1, ROWS], F32, tag="musq")
                nc.vector.tensor_mul(out=musq[:], in0=mun[:], in1=mun[:])
                var = smp.tile([1, ROWS], F32, tag="var")
                nc.vector.tensor_sub(out=var[:], in0=e2[:], in1=musq[:])
                std = smp.tile([1, ROWS], F32, tag="std")
                nc.scalar.activation(out=std[:], in_=var[:], func=AF.Sqrt,
                                     bias=eps_t[:], scale=1.0)
                cc = smp.tile([1, 2 * ROWS], FR, tag="cc")
                with nc.allow_low_precision(reason="rstd feeds fp32r matmul"):
                    nc.vector.reciprocal(out=cc[:, 0:ROWS], in_=std[:])
                nc.vector.tensor_mul(
                    out=cc[:, ROWS:2 * ROWS], in0=mun[:],
                    in1=cc[:, 0:ROWS].bitcast(F32),
                )
                ps_bc = bcp.tile([128, 2 * ROWS], F32, tag="ps_bc")
                nc.tensor.matmul(ps_bc[:], ones_row[:], cc[:], start=True, stop=True)
                with tc.tile_pool(name=f"{tag}_tmp", bufs=2) as tmpp:
                    for k in range(HC):
                        tmp = tmpp.tile([128, ROWS], F32, tag="tmp")
                        nc.vector.tensor_mul(
                            out=tmp[:], in0=zs[k][:].bitcast(F32),
                            in1=ps_bc[:, 0:ROWS],
                        )
                        nc.vector.tensor_add(
                            out=tmp[:], in0=tmp[:], in1=ps_bc[:, ROWS:2 * ROWS]
                        )
                        nc.scalar.activation(
                            out=out[k][:], in_=tmp[:], func=AF.Gelu,
                            bias=beta_cols[:, k:k + 1], scale=g_cols[:, k:k + 1],
                        )
            return out

        def matmul_6x6(w_tiles, rhs_tiles, bias_cols, out_tag, psname):
            """out_T[j] = w.T @ rhs + bias; 6 psum tiles rotating, immediate
            eviction; returns 6 fr tiles [128, ROWS]."""
            out = [acts.tile([128, ROWS], FR, tag=f"{out_tag}{j}", name=f"{out_tag}{j}") for j in range(HC)]
            with tc.tile_pool(name=psname, bufs=3, space="PSUM") as psp:
                for j in range(HC):
                    pj = psp.tile([128, ROWS], F32, tag="ps")
                    for k in range(HC):
                        nc.tensor.matmul(
                            pj[:], w_tiles[k][:, 128 * j:128 * (j + 1)],
                            rhs_tiles[k][:],
                            start=(k == 0), stop=(k == HC - 1),
                        )
                    nc.scalar.activation(
                        out=out[j][:], in_=pj[:], func=AF.Identity,
                        bias=bias_cols[:, j:j + 1], scale=1.0,
                    )
            return out

        tap_chunks("d_repsT", repsT)

        # ============ Stage 2: encoder ============
        z_enc = matmul_6x6(wenc_t, repsT, benc_t, "z", "ps_enc")
        tap_chunks("d_z", z_enc)
        x_T = ln_gelu(z_enc, genc_t, beenc_t, "enc")
        tap_chunks("d_xT", x_T)

        # qk weights (q part prescaled by 1/sqrt(HD) on host); v weights
        wqk_t = [wpool.tile([128, 2 * H], FR, tag=f"wqk{k}", name=f"wqk{k}") for k in range(HC)]
        for k in range(HC):
            nc.sync.dma_start(out=wqk_t[k][:], in_=wqk[128 * k:128 * (k + 1), :])
        wv_t = [wmid.tile([128, H], FR, tag=f"wmid{k}", name=f"wvt{k}") for k in range(HC)]
        for k in range(HC):
            nc.sync.dma_start(out=wv_t[k][:], in_=wv[128 * k:128 * (k + 1), :])

        # ============ Stage 3: q/k per head, v natural + per-sample ============
        qk_h = [acts.tile([HD, ROWS], FR, tag=f"qk{i}", name=f"qk{i}") for i in range(2 * NH)]
        with tc.tile_pool(name="ps_qk", bufs=4, space="PSUM") as ps_qk:
            for i in range(2 * NH):  # 0..7 q heads, 8..15 k heads
                pq = ps_qk.tile([HD, ROWS], F32, tag="ps_qk")
                c0 = i * HD  # column offset in wqk
                for k in range(HC):
                    nc.tensor.matmul(
                        pq[:], wqk_t[k][:, c0:c0 + HD], x_T[k][:],
                        start=(k == 0), stop=(k == HC - 1),
                    )
                nc.scalar.activation(
                    out=qk_h[i][:], in_=pq[:], func=AF.Identity,
                    bias=bqk_t[:, i:i + 1], scale=1.0,
                )

        tap("d_q0", qk_h[0])
        tap("d_k0", qk_h[NH])
        v_b = [acts.tile([S, H], FR, tag=f"vb{b}", name=f"vb{b}") for b in range(BC)]
        with tc.tile_pool(name="ps_v", bufs=2, space="PSUM") as ps_v, \
             tc.tile_pool(name="vnat", bufs=2) as vnat:
            for ih in range(2):
                pv = ps_v.tile([128, H], F32, tag="ps_v")
                for k in range(HC):
                    for c0, c1 in ((0, 512), (512, 768)):
                        nc.tensor.matmul(
                            pv[:, c0:c1],
                            x_T[k][:, 128 * ih:128 * (ih + 1)],
                            wv_t[k][:, c0:c1],
                            start=(k == 0), stop=(k == HC - 1),
                        )
                vn = vnat.tile([128, H], FR, tag="nat2")
                nc.vector.tensor_add(out=vn[:], in0=pv[:], in1=vbias[:])
                for b4 in range(4):
                    b = ih * 4 + b4
                    nc.sync.dma_start(
                        out=v_b[b][:], in_=vn[32 * b4:32 * (b4 + 1), :]
                    )

        # ============ Stage 4: attention ============
        # scores per sample: [32, 256] = 8 heads along free axis; softmax with
        # 3D-AP reduces + step-0 broadcast; probs transposed per 32x32 block
        # on the DVE (= per-head transpose).
        pT = [None] * BC
        with tc.tile_pool(name="ps_sc", bufs=4, space="PSUM") as ps_sc, \
             tc.tile_pool(name="smax", bufs=4) as smax, \
             tc.tile_pool(name="ptp", bufs=1) as ptp:
            for b in range(BC):
                psc = ps_sc.tile([S, ROWS], F32, tag="ps_sc")
                for n in range(NH):
                    nc.tensor.matmul(
                        psc[:, 32 * n:32 * (n + 1)],
                        qk_h[n][:, 32 * b:32 * (b + 1)],
                        qk_h[NH + n][:, 32 * b:32 * (b + 1)],
                        start=True, stop=True,
                    )
                psc3 = psc[:].rearrange("p (n t) -> p n t", t=S)
                mx = smax.tile([S, NH], F32, tag="mx")
                nc.vector.reduce_max(out=mx[:], in_=psc3,
                                     axis=mybir.AxisListType.X)
                ex = smax.tile([S, ROWS], F32, tag="ex")
                nc.vector.tensor_sub(
                    out=ex[:].rearrange("p (n t) -> p n t", t=S),
                    in0=psc3,
                    in1=mx[:].to_broadcast([S, NH, S]),
                )
                nc.scalar.activation(out=ex[:], in_=ex[:], func=AF.Exp,
                                     bias=0.0, scale=1.0)
                sm = smax.tile([S, NH], F32, tag="sm")
                nc.vector.reduce_sum(out=sm[:],
                                     in_=ex[:].rearrange("p (n t) -> p n t", t=S),
                                     axis=mybir.AxisListType.X)
                rr = smax.tile([S, NH], F32, tag="rr")
                nc.vector.reciprocal(out=rr[:], in_=sm[:])
                pe = smax.tile([S, ROWS], F32, tag="pe")
                nc.vector.tensor_mul(
                    out=pe[:].rearrange("p (n t) -> p n t", t=S),
                    in0=ex[:].rearrange("p (n t) -> p n t", t=S),
                    in1=rr[:].to_broadcast([S, NH, S]),
                )
                ptf = smax.tile([S, ROWS], F32, tag="ptf")
                nc.vector.transpose(out=ptf[:], in_=pe[:])
                pT[b] = ptp.tile([S, ROWS], FR, tag=f"pT{b}", name=f"pT{b}")
                nc.vector.tensor_copy(out=pT[b][:], in_=ptf[:])

        tap("d_v0", v_b[0])
        tap("d_pT0", pT[0])

        # out-proj weights per head rows [96, H] (reuse wv slots)
        wout_t = [wmid.tile([HD, H], FR, tag=f"wmid{n}", name=f"wout{n}") for n in range(NH)]
        for n in range(NH):
            nc.sync.dma_start(out=wout_t[n][:], in_=wout[HD * n:HD * (n + 1), :])
        # cls1 weights reuse encoder weight slots
        wcls1_t = [wpool.tile([128, H], FR, tag=f"wsq{k}", name=f"wsq{k}") for k in range(HC)]
        for k in range(HC):
            nc.sync.dma_start(out=wcls1_t[k][:], in_=wcls1[128 * k:128 * (k + 1), :])

        # AV: per-head oT [96, ROWS] (reuse q-head slots; q/k dead after scores)
        oT = [acts.tile([HD, ROWS], FR, tag=f"qk{n}", name=f"oT{n}") for n in range(NH)]
        with tc.tile_pool(name="ps_av", bufs=4, space="PSUM") as ps_av:
            for n in range(NH):
                po = ps_av.tile([HD, ROWS], F32, tag="ps_av")
                for b in range(BC):
                    nc.tensor.matmul(
                        po[:, 32 * b:32 * (b + 1)],
                        v_b[b][:, HD * n:HD * (n + 1)],
                        pT[b][:, 32 * n:32 * (n + 1)],
                        start=True, stop=True,
                    )
                nc.scalar.copy(out=oT[n][:], in_=po[:])

        # ============ Stage 5: out-proj + classifier ============
        attT = [acts.tile([128, ROWS], FR, tag=f"ra{j}", name=f"attT{j}") for j in range(HC)]
        with tc.tile_pool(name="ps_at", bufs=3, space="PSUM") as ps_at:
            for j in range(HC):
                pa = ps_at.tile([128, ROWS], F32, tag="ps_at")
                for n in range(NH):
                    nc.tensor.matmul(
                        pa[:], wout_t[n][:, 128 * j:128 * (j + 1)], oT[n][:],
                        start=(n == 0), stop=(n == NH - 1),
                    )
                nc.scalar.activation(
                    out=attT[j][:], in_=pa[:], func=AF.Identity,
                    bias=bout_t[:, j:j + 1], scale=1.0,
                )

        tap("d_oT0", oT[0])
        tap_chunks("d_attT", attT)
        z_cls = matmul_6x6(wcls1_t, attT, bcls1_t, "z", "ps_c1")
        tap_chunks("d_zc", z_cls)
        h_T = ln_gelu(z_cls, gcls_t, becls_t, "cls")

        tap_chunks("d_hT", h_T)
        wcls2_t = [wpool.tile([128, 2], FR, tag=f"wc2{k}", name=f"wc2{k}") for k in range(HC)]
        for k in range(HC):
            nc.sync.dma_start(out=wcls2_t[k][:], in_=wcls2[128 * k:128 * (k + 1), :])

        # ============ Stage 6: logits + CE ============
        with tc.tile_pool(name="ps_lg", bufs=1, space="PSUM") as ps_lg, \
             tc.tile_pool(name="ps_lt", bufs=2, space="PSUM") as ps_lt, \
             tc.tile_pool(name="ps_ls", bufs=1, space="PSUM") as ps_ls, \
             tc.tile_pool(name="cep", bufs=1) as cep:
            plg = ps_lg.tile([2, ROWS], F32, tag="ps_lg")
            for k in range(HC):
                nc.tensor.matmul(plg[:], wcls2_t[k][:], h_T[k][:],
                                 start=(k == 0), stop=(k == HC - 1))
            lgT = cep.tile([2, ROWS], F32, tag="lgT")
            nc.scalar.activation(out=lgT[:], in_=plg[:], func=AF.Identity,
                                 bias=bcls2_t[:, 0:1], scale=1.0)
            ps_loss = ps_ls.tile([2, 2], F32, tag="ps_loss")
            nll2 = cep.tile([128, 2], FR, tag="nll2")
            for t in range(2):
                ptl = ps_lt.tile([128, 2], F32, tag="ps_lt")
                nc.tensor.transpose(
                    ptl[:], lgT[:, 128 * t:128 * (t + 1)],
                    ident[0:2, 0:2].bitcast(F32),
                )
                lg = cep.tile([128, 2], F32, tag=f"lg{t}")
                nc.scalar.copy(out=lg[:], in_=ptl[:])
                nc.sync.dma_start(out=logits_out[128 * t:128 * (t + 1), :], in_=lg[:])
                mx = cep.tile([128, 1], F32, tag=f"cmx{t}")
                nc.vector.reduce_max(out=mx[:], in_=lg[:], axis=mybir.AxisListType.X)
                nm = cep.tile([128, 1], F32, tag=f"cnm{t}")
                nc.vector.tensor_scalar_mul(nm[:], mx[:], -1.0)
                ex = cep.tile([128, 2], F32, tag=f"cex{t}")
                sm = cep.tile([128, 1], F32, tag=f"csm{t}")
                nc.scalar.activation(out=ex[:], in_=lg[:], func=AF.Exp,
                                     bias=nm[:], scale=1.0, accum_out=sm[:])
                ln_ = cep.tile([128, 1], F32, tag=f"cln{t}")
                nc.scalar.activation(out=ln_[:], in_=sm[:], func=AF.Ln,
                                     bias=0.0, scale=1.0)
                lse = cep.tile([128, 1], F32, tag=f"clse{t}")
                nc.vector.tensor_sub(out=lse[:], in0=ln_[:], in1=nm[:])
                df = cep.tile([128, 1], F32, tag=f"cdf{t}")
                nc.vector.tensor_sub(out=df[:], in0=lg[:, 1:2], in1=lg[:, 0:1])
                nc.vector.tensor_mul(out=df[:], in0=df[:], in1=lab_t[:, t:t + 1])
                nc.vector.tensor_add(out=df[:], in0=df[:], in1=lg[:, 0:1])
                nll = cep.tile([128, 1], F32, tag=f"cnl{t}")
                nc.vector.tensor_sub(out=nll[:], in0=lse[:], in1=df[:])
                nc.vector.tensor_mul(
                    out=nll2[:, t:t + 1], in0=nll[:], in1=val_t[:, t:t + 1]
                )
            nc.tensor.matmul(ps_loss[:], ones_col[:], nll2[:],
                             start=True, stop=True)
            lsb = cep.tile([1, 2], F32, tag="lsb")
            nc.scalar.copy(out=lsb[:], in_=ps_loss[0:1, :])
            nc.sync.dma_start(out=loss_out[:], in_=lsb[:])

    nc.finalize()
    return nc


def _host_prep(inputs):
    ids = np.asarray(inputs["input_ids"])
    amask = np.asarray(inputs["attention_mask"]).astype(np.float32)
    labels = np.asarray(inputs["labels"])
    seq = np.ascontiguousarray(np.asarray(inputs["sequence_output"], dtype=np.float32))

    # segment-mean pooling matrix, pre-normalized (matches reference exactly:
    # sum(mask * seq) / (sum(mask) + 1e-10))
    sep = ids == SEP
    pos = np.argsort(~sep, axis=1, kind="stable")[:, :S]
    pos = np.sort(pos, axis=1)
    starts = np.concatenate([np.zeros((B, 1), pos.dtype), pos[:, :S - 1]], axis=1)
    ends = pos
    t = np.arange(L)
    seg = (t[None, None, :] >= starts[:, :, None]) & (t[None, None, :] <= ends[:, :, None])
    m = seg.astype(np.float32) * amask[:, None, :]
    denom = m.sum(axis=2, keepdims=True) + 1e-10
    mT = np.ascontiguousarray((m / denom).transpose(0, 2, 1))  # [B, L, S]

    isq = 1.0 / np.sqrt(np.float32(HD))
    wqk_h = np.concatenate(
        [np.asarray(inputs["in_proj_W"][:H]).T * isq,
         np.asarray(inputs["in_proj_W"][H:2 * H]).T], axis=1
    )
    b_qk_h = np.concatenate(
        [np.asarray(inputs["in_proj_b"][:H]) * isq,
         np.asarray(inputs["in_proj_b"][H:2 * H])]
    )

    shared = dict(
        wenc=np.ascontiguousarray(np.asarray(inputs["enc_W"]).T, dtype=np.float32),
        wqk=np.ascontiguousarray(wqk_h, dtype=np.float32),
        wv=np.ascontiguousarray(np.asarray(inputs["in_proj_W"][2 * H:]).T, dtype=np.float32),
        wout=np.ascontiguousarray(np.asarray(inputs["out_proj_W"]).T, dtype=np.float32),
        wcls1=np.ascontiguousarray(np.asarray(inputs["cls_W1"]).T, dtype=np.float32),
        wcls2=np.ascontiguousarray(np.asarray(inputs["cls_W2"]).T, dtype=np.float32),
        ident=np.eye(128, dtype=np.float32),
        ones_col=np.ones((128, 2), dtype=np.float32),
        ones_row=np.ones((1, 128), dtype=np.float32),
        vbias_bc=np.ascontiguousarray(
            np.broadcast_to(np.asarray(inputs["in_proj_b"][2 * H:]), (128, H)),
            dtype=np.float32,
        ),
        b_enc=np.asarray(inputs["enc_b"], np.float32).reshape(H, 1),
        b_qk=np.asarray(b_qk_h, np.float32).reshape(2 * H, 1),
        b_out=np.asarray(inputs["out_proj_b"], np.float32).reshape(H, 1),
        b_cls1=np.asarray(inputs["cls_b1"], np.float32).reshape(H, 1),
        b_cls2=np.asarray(inputs["cls_b2"], np.float32).reshape(2, 1),
        g_enc=np.asarray(inputs["enc_ln_g"], np.float32).reshape(H, 1),
        be_enc=np.asarray(inputs["enc_ln_b"], np.float32).reshape(H, 1),
        g_cls=np.asarray(inputs["cls_ln_g"], np.float32).reshape(H, 1),
        be_cls=np.asarray(inputs["cls_ln_b"], np.float32).reshape(H, 1),
    )

    valid = labels != -100
    safe = np.where(valid, labels, 0).astype(np.float32)

    in_maps = []
    for c in range(NCORES):
        b0, b1 = c * BC, (c + 1) * BC
        in_maps.append(dict(
            shared,
            seq=np.ascontiguousarray(seq[b0:b1].reshape(SEQR, H)),
            poolT=np.ascontiguousarray(mT[b0:b1].reshape(SEQR, S)),
            lab=np.ascontiguousarray(safe[b0:b1].reshape(ROWS, 1)),
            val=np.ascontiguousarray(
                valid[b0:b1].reshape(ROWS, 1).astype(np.float32)
            ),
        ))
    n_valid = int(valid.sum())
    return in_maps, n_valid


def kernel(**inputs):
    global _PROGRAM
    if _PROGRAM is None:
        _PROGRAM = _build_program()
    nc = _PROGRAM
    in_maps, n_valid = _host_prep(inputs)
    res = run_bass_kernel_spmd(nc, in_maps, list(range(NCORES)))
    logits = np.stack(
        [res.results[c]["logits_out"] for c in range(NCORES)]
    ).reshape(B, S, 2).astype(np.float32)
    total = sum(float(res.results[c]["loss_out"].sum()) for c in range(NCORES))
    loss = np.float32(total / max(n_valid, 1))
    return loss, logits
